# revision 26
# baseline (speedup 1.0000x reference)
"""Bass/Tile TRN2 kernel for nn_Attn (Bahdanau-style attention scores).

Reference computation (B=32, S=2048, H=1024):
    enc    = transpose(encoder_outputs, (1,0,2))            # [B,S,H]
    cat    = concat([hidden[:,None,:] broadcast, enc], -1)  # [B,S,2H]
    energy = tanh(cat @ W.T + b)                            # [B,S,H]
    scores = energy @ v[0]                                  # [B,S]
    attn   = softmax(scores, axis=-1)[:, None, :]           # [B,1,S]

Distribution: data-parallel over batch. 8 cores x 4 batches each.
W/b/v replicated. Host does layout-only prep (slices + transposes +
dtype casts, no arithmetic): enc arrives per-core already k-major and
partition-blocked so every DMA descriptor is 16KB contiguous; W2^T is
ho-blocked; W1^T/hidden are bf16 (their error is seq-constant-ish and
mostly cancels in the softmax).

Per-core algorithm (big matmuls in float32r via bitcast: ~11-bit
effective mantissa at 1 cycle/row for moving dim >= 256):
    warmup: ~7 dummy MMs on zero scratch flip the PE HAM clock-gate
            to 8/8 before the real data lands.
    u      = W1^T.T @ hidden^T + b (bf16 MMs, 2MB DMA) + 8 tiny PE
             transposes -> u_all [128, 8ho, 4b]
    per chunk (sc, bi): encT [128, 8, 512] DMA'd (pre-blocked)
      T^T[ho]  = sum_kj wt[ho][kj].T @ encT[kj]     (PSUM accum, 8 mm)
      et       = tanh(T^T + u[:,ho,bi])             (ACT, bias column)
      acc     += et * v[ho]                         (DVE fused mul-add)
      pscore[bi,:] += ones-masked partition-sum of acc   (1 matmul)
      after bi=3: nmx_sc = -max(pscore); draft exp(s + nmx_sc) into
      attn_sb with running chunk sum (online softmax, ACT accum)
    tail: m = max over chunk maxes; f = exp(cmx - m); T = sum f*ssum;
          phi = f/T; attn[:, sc] *= phi[sc] (8 on DVE + 8 on ACT); DMA.
"""

import numpy as np

B, S, H = 32, 2048, 1024
NCORES = 8
BPC = B // NCORES          # batches per core
SC = 512                   # s-chunk (matmul moving size)
NSC = S // SC              # chunks per batch
KB = H // 128              # 128-blocks along one H
P = 128
NWARM = 7                  # dummy warm-up matmuls

_compiled = {}


def _build():
    import concourse.bass as bass
    import concourse.mybir as mybir
    from concourse import bacc, tile, masks

    f32 = mybir.dt.float32
    f32r = mybir.dt.float32r
    bf16 = mybir.dt.bfloat16
    Tanh = mybir.ActivationFunctionType.Tanh
    Exp = mybir.ActivationFunctionType.Exp
    Copy = mybir.ActivationFunctionType.Copy
    Mult = mybir.AluOpType.mult
    Add = mybir.AluOpType.add
    Min = mybir.AluOpType.min
    Bypass = mybir.AluOpType.bypass

    nc = bacc.Bacc("TRN2", target_bir_lowering=False, debug=False,
                   num_devices=NCORES)

    # host supplies pre-transposed/blocked layouts (layout + dtype only):
    #   enc_t: [NSC, BPC, P, KB, SC]  enc^T blocked; 16KB/partition DMAs
    #   wt2b:  [KB(ho), P, KB(kj), P] W2^T ho-col-blocked (f32r bitcast)
    #   w1cb:  [KB(ho), P, KB(kj), P] W1^T ho-col-blocked, bf16
    #   hidt:  [P, KB, BPC]           hidden^T blocked, bf16
    #   biast: [P, KB]                b blocked
    #   vt:    [P, KB]                v blocked
    enc_d = nc.declare_dram_parameter("enc_t", [NSC, BPC, P, KB, SC], f32r,
                                      isOutput=False)
    wt_d = nc.declare_dram_parameter("wt2b", [KB, P, KB, P], f32r,
                                     isOutput=False)
    w1_d = nc.declare_dram_parameter("w1cb", [KB, P, KB, P], bf16,
                                     isOutput=False)
    hidt_d = nc.declare_dram_parameter("hidt", [P, KB, BPC], bf16,
                                       isOutput=False)
    biast_d = nc.declare_dram_parameter("biast", [P, KB], f32, isOutput=False)
    vt_d = nc.declare_dram_parameter("vt", [P, KB], f32, isOutput=False)
    out_d = nc.declare_dram_parameter("attn", [BPC, S], f32, isOutput=True)

    with tile.TileContext(nc) as tc:
        import contextlib
        with contextlib.ExitStack() as ctx:
            const = ctx.enter_context(tc.tile_pool(name="const", bufs=1))
            persist = ctx.enter_context(tc.tile_pool(name="persist", bufs=1))
            wnat = ctx.enter_context(tc.tile_pool(name="wnat", bufs=2))
            encp = ctx.enter_context(tc.tile_pool(name="encp", bufs=3))
            etp = ctx.enter_context(tc.tile_pool(name="etp", bufs=3))
            accp = ctx.enter_context(tc.tile_pool(name="accp", bufs=2))
            ps_m = ctx.enter_context(
                tc.tile_pool(name="ps_m", bufs=5, space="PSUM"))
            ps_s = ctx.enter_context(
                tc.tile_pool(name="ps_s", bufs=3, space="PSUM"))

            # ---------- PE warm-up: dummy MMs on zeroed scratch ----------
            scratch = const.tile([P, SC], f32, tag="scratch")
            nc.gpsimd.memset(scratch[:], 0.0)
            pdum = ps_m.tile([P, SC], f32, tag="pm", name="pdum")
            for i in range(NWARM):
                nc.tensor.matmul(pdum[:],
                                 scratch[:, 0:P].bitcast(f32r),
                                 scratch[:].bitcast(f32r),
                                 start=(i == 0), stop=(i == NWARM - 1))

            # ---------- small constants (sync ring, first) ----------
            hidT = const.tile([P, KB, BPC], bf16, tag="hidT")
            nc.sync.dma_start(hidT[:], hidt_d[:])
            biasT = const.tile([P, KB], f32, tag="biasT")
            nc.sync.dma_start(biasT[:], biast_d[:])
            vT = const.tile([P, KB], f32, tag="vT")
            nc.sync.dma_start(vT[:], vt_d[:])

            # W2^T ho-column blocks, in ho-consumption order, split
            # across the two HWDGE rings (sync: even, scalar: odd).
            wt_ho = []
            for ho in range(KB):
                t = persist.tile([P, KB, P], f32r, tag=f"wt{ho}",
                                 name=f"wt{ho}")
                eng = nc.sync if ho % 2 == 0 else nc.scalar
                eng.dma_start(t[:], wt_d[ho])
                wt_ho.append(t)

            # ---------- enc chunk prefetch ----------
            chunks = [(sc, bi) for sc in range(NSC) for bi in range(BPC)]
            PREFETCH = 3
            pending = {}

            # chunk 0 (per-kj, gpsimd ring — interleaved with the W1
            # blocks below so the PE gets work every ~1us at startup)
            enc0 = encp.tile([P, KB, SC], f32r, tag="enc", name="enc0_0")
            # chunk 1 (per-kj, sync ring behind the even wt blocks)
            enc1 = encp.tile([P, KB, SC], f32r, tag="enc", name="enc0_1")
            for kj in range(KB):
                nc.sync.dma_start(enc1[:, kj, :], enc_d[0, 1, :, kj])
            # chunk 2 (scalar ring behind the odd wt blocks)
            enc2 = encp.tile([P, KB, SC], f32r, tag="enc", name="enc0_2")
            nc.scalar.dma_start(enc2[:], enc_d[0, 2])

            # W1^T bf16 ho-column blocks for the u matmuls, interleaved
            # with chunk-0 pieces on the gpsimd ring (late blocks last).
            w1c = []
            for ho in range(KB):
                t = wnat.tile([P, KB, P], bf16, tag=f"w1c{ho}", bufs=1,
                              name=f"w1c{ho}")
                w1c.append(t)
            nc.gpsimd.dma_start(w1c[0][:], w1_d[0])
            nc.gpsimd.dma_start(w1c[1][:], w1_d[1])
            for kj in range(KB):
                nc.gpsimd.dma_start(enc0[:, kj, :], enc_d[0, 0, :, kj])
                if kj % 2 == 0 and kj // 2 + 2 < KB:
                    nc.gpsimd.dma_start(w1c[kj // 2 + 2][:],
                                        w1_d[kj // 2 + 2])
            for ho in range(KB // 2 + 2, KB):
                nc.gpsimd.dma_start(w1c[ho][:], w1_d[ho])

            pending[0] = enc0
            pending[1] = enc1
            pending[2] = enc2

            def fetch(idx):
                sc, bi = chunks[idx]
                t = encp.tile([P, KB, SC], f32r, tag="enc",
                              name=f"enc{sc}_{bi}")
                nc.gpsimd.dma_start(t[:], enc_d[sc, bi])
                return t

            # masked-ones stationaries: mask4[:, c, bi] = 1.0 iff c == bi.
            ones = const.tile([P, 1], f32, tag="ones")
            nc.gpsimd.memset(ones[:], 1.0)
            mask4 = const.tile([P, BPC, BPC], f32r, tag="mask4")
            zt = wnat.tile([P, BPC * BPC], f32, tag="zero", bufs=1)
            nc.gpsimd.memset(zt[:], 0.0)
            nc.vector.tensor_copy(
                mask4[:].rearrange("p a b -> p (a b)"), zt[:])
            for bi in range(BPC):
                nc.vector.tensor_copy(mask4[:, bi, bi:bi + 1], ones[:])

            # ---------- u^T = W1 @ hidden^T (+ bias), bf16 ----------
            # stationary = W1^T column block [128k, 128h], moving = hidT
            # [128k, 4b]; psum partition dim is h, so no transposes.
            u_all = const.tile([P, KB, BPC], f32, tag="u")
            for ho in range(KB):
                pu = ps_s.tile([P, BPC], f32, tag="ps_small", name=f"pu{ho}")
                for kj in range(KB):
                    nc.tensor.matmul(
                        pu[:], w1c[ho][:, kj, :], hidT[:, kj, :],
                        start=(kj == 0), stop=(kj == KB - 1))
                nc.vector.tensor_scalar_add(
                    u_all[:, ho, :], pu[:], biasT[:, ho:ho + 1])

            # second warm-up burst: bridges the gap between the u phase
            # and the first main chunk so the PE clock-gate stays warm
            for i in range(8):
                nc.tensor.matmul(pdum[:],
                                 scratch[:, 0:P].bitcast(f32r),
                                 scratch[:].bitcast(f32r),
                                 start=(i == 0), stop=(i == 7))

            # ---------- softmax state ----------
            attn_sb = persist.tile([BPC, S], f32, tag="attn")
            nmx = const.tile([BPC, NSC], f32, tag="nmx")     # -chunk max
            ssum = const.tile([BPC, NSC], f32, tag="ssum")   # chunk expsum
            mn = const.tile([BPC, 1], f32, tag="mn")         # -global max
            f = const.tile([BPC, NSC], f32, tag="f")         # exp(cmx - m)
            tt = const.tile([BPC, NSC], f32, tag="tt")
            t014 = const.tile([BPC, 1], f32, tag="t014")

            # ---------- main loop ----------
            for idx, (sc, bi) in enumerate(chunks):
                s0 = sc * SC
                encT = pending.pop(idx)
                if idx + PREFETCH < len(chunks):
                    pending[idx + PREFETCH] = fetch(idx + PREFETCH)

                if sc == NSC - 1 and bi == 0:
                    # precompute everything the last chunk's softmax
                    # needs, off the critical path. The last chunk's
                    # exp uses the chunks-0..14 max as stabilizer (the
                    # softmax identity is exact for any stabilizer).
                    nc.vector.tensor_reduce(
                        out=mn[:], in_=nmx[:, 0:NSC - 1], op=Min,
                        axis=mybir.AxisListType.X)
                    nc.scalar.activation(
                        f[:, 0:NSC - 1], nmx[:, 0:NSC - 1], Exp,
                        bias=mn[:], scale=-1.0)
                    nc.gpsimd.memset(f[:, NSC - 1:NSC], 1.0)
                    nc.vector.scalar_tensor_tensor(
                        tt[:, 0:NSC - 1], f[:, 0:NSC - 1], 1.0,
                        ssum[:, 0:NSC - 1],
                        op0=Bypass, op1=Mult, accum_out=t014[:])

                acc = accp.tile([P, SC], f32r, tag="acc",
                                name=f"acc{sc}_{bi}")
                for ho in range(KB):
                    pm = ps_m.tile([P, SC], f32, tag="pm",
                                   name=f"pm{sc}_{bi}_{ho}")
                    for kj in range(KB):
                        nc.tensor.matmul(
                            pm[:],
                            wt_ho[ho][:, kj, :],
                            encT[:, kj, :],
                            start=(kj == 0), stop=(kj == KB - 1))

                    et = etp.tile([P, SC], f32, tag="et",
                                  name=f"et{sc}_{bi}_{ho}")
                    nc.scalar.activation(
                        et[:], pm[:], Tanh,
                        bias=u_all[:, ho, bi:bi + 1], scale=1.0)
                    if ho == 0:
                        nc.vector.tensor_scalar_mul(
                            acc[:], et[:], vT[:, 0:1])
                    else:
                        nc.vector.scalar_tensor_tensor(
                            acc[:], et[:], vT[:, ho:ho + 1], acc[:],
                            op0=Mult, op1=Add)

                if bi == 0:
                    pscore = ps_s.tile([BPC, SC], f32, tag="ps_small",
                                       name=f"pscore{sc}")
                nc.tensor.matmul(
                    pscore[:], mask4[:, :, bi],
                    acc[:],
                    start=(bi == 0), stop=(bi == BPC - 1))
                if bi == BPC - 1:
                    # online softmax: draft exp(s - m_sc) + running sum
                    if sc < NSC - 1:
                        nc.vector.reduce_max(
                            nmx[:, sc:sc + 1], pscore[:],
                            axis=mybir.AxisListType.X, negate=True)
                        nc.scalar.activation(
                            attn_sb[:, s0:s0 + SC], pscore[:], Exp,
                            bias=nmx[:, sc:sc + 1], scale=1.0,
                            accum_out=ssum[:, sc:sc + 1])
                    else:
                        # last chunk: precomputed stabilizer, no reduce
                        nc.scalar.activation(
                            attn_sb[:, s0:s0 + SC], pscore[:], Exp,
                            bias=mn[:], scale=1.0,
                            accum_out=ssum[:, sc:sc + 1])

            # ---------- softmax tail ----------
            # T = sum f*ssum = t014 + ssum[15]; phi = f / T
            tsum = const.tile([BPC, 1], f32, tag="tsum")
            nc.vector.scalar_tensor_tensor(
                tsum[:], t014[:], 1.0, ssum[:, NSC - 1:NSC],
                op0=Bypass, op1=Add)
            rs = const.tile([BPC, 1], f32, tag="rs")
            nc.vector.reciprocal(rs[:], tsum[:])
            phi = const.tile([BPC, NSC], f32, tag="phi")
            nc.vector.tensor_scalar_mul(phi[:], f[:], rs[:])
            # rescale chunks: split across DVE and ACT queues
            HALF = NSC // 2
            for sc in range(NSC):
                s0 = sc * SC
                if sc % HALF < HALF // 2:
                    nc.vector.tensor_scalar_mul(
                        attn_sb[:, s0:s0 + SC], attn_sb[:, s0:s0 + SC],
                        phi[:, sc:sc + 1])
                else:
                    nc.scalar.activation(
                        attn_sb[:, s0:s0 + SC], attn_sb[:, s0:s0 + SC],
                        Copy, bias=0.0, scale=phi[:, sc:sc + 1])
            nc.scalar.dma_start(out_d[:], attn_sb[:])

    nc.compile()
    return nc


def _get_nc():
    if "nc" not in _compiled:
        _compiled["nc"] = _build()
    return _compiled["nc"]


def _make_in_maps(hidden, encoder_outputs, W, b, v):
    import ml_dtypes

    hidden = np.ascontiguousarray(hidden, dtype=np.float32)
    encoder_outputs = np.ascontiguousarray(encoder_outputs, dtype=np.float32)
    W = np.asarray(W, dtype=np.float32)
    b = np.asarray(b, dtype=np.float32).reshape(H)
    v = np.asarray(v, dtype=np.float32).reshape(H)

    # layout-only host prep (replicated across cores)
    WT = W.T                                                 # [2H, H]
    # W2^T blocked [ho, p(k), kj, c(h)] -> contiguous per-partition DMAs
    wt2b = np.ascontiguousarray(
        WT[H:].reshape(KB, P, KB, P).transpose(2, 1, 0, 3))  # [ho,p,kj,c]
    w1cb = np.ascontiguousarray(
        WT[:H].reshape(KB, P, KB, P).transpose(2, 1, 0, 3)
    ).astype(ml_dtypes.bfloat16)                              # [ho,p,kj,c]
    biast = np.ascontiguousarray(b.reshape(KB, P).T)          # [128, 8]
    vt = np.ascontiguousarray(v.reshape(KB, P).T)             # [128, 8]

    in_maps = []
    for c in range(NCORES):
        bs = slice(c * BPC, (c + 1) * BPC)
        hidt = np.ascontiguousarray(
            hidden[bs].T.reshape(KB, P, BPC).transpose(1, 0, 2)
        ).astype(ml_dtypes.bfloat16)                          # [128,8,4]
        # enc blocked [sc, bi, p, kj, s]; 16KB contiguous per partition
        sl = encoder_outputs[:, bs, :]                        # [S,4,H]
        enc_t = np.empty((NSC, BPC, P, KB, SC), np.float32)
        for sc in range(NSC):
            blk = sl[sc * SC:(sc + 1) * SC]                   # [512,4,1024]
            enc_t[sc] = (blk.transpose(1, 2, 0)               # [4,1024,512]
                         .reshape(BPC, KB, P, SC)
                         .transpose(0, 2, 1, 3))              # [4,128,8,512]
        in_maps.append({
            "enc_t": enc_t,
            "wt2b": wt2b,
            "w1cb": w1cb,
            "hidt": hidt,
            "biast": biast,
            "vt": vt,
        })
    return in_maps


def kernel(hidden, encoder_outputs, W, b, v):
    from concourse.bass_utils import run_bass_kernel_spmd

    nc = _get_nc()
    in_maps = _make_in_maps(hidden, encoder_outputs, W, b, v)
    res = run_bass_kernel_spmd(nc, in_maps, list(range(NCORES)))
    _compiled["last_result"] = res
    attn = np.concatenate(
        [res.results[c]["attn"] for c in range(NCORES)], axis=0)  # [B, S]
    return attn[:, None, :].astype(np.float32)


if __name__ == "__main__":
    rng = np.random.default_rng(0)
    inputs = {
        "hidden": rng.standard_normal((B, H)).astype(np.float32),
        "encoder_outputs": rng.standard_normal((S, B, H)).astype(np.float32),
        "W": (rng.standard_normal((H, 2 * H)) / np.sqrt(2 * H)).astype(np.float32),
        "b": (rng.standard_normal(H) * 0.01).astype(np.float32),
        "v": rng.standard_normal((1, H)).astype(np.float32),
    }
    out = kernel(**inputs)
    print("out", out.shape, out.dtype, out.sum())


# revision 29
# speedup vs baseline: 1.0248x; 1.0248x over previous
"""Bass/Tile TRN2 kernel for nn_Attn (Bahdanau-style attention scores).

Reference computation (B=32, S=2048, H=1024):
    enc    = transpose(encoder_outputs, (1,0,2))            # [B,S,H]
    cat    = concat([hidden[:,None,:] broadcast, enc], -1)  # [B,S,2H]
    energy = tanh(cat @ W.T + b)                            # [B,S,H]
    scores = energy @ v[0]                                  # [B,S]
    attn   = softmax(scores, axis=-1)[:, None, :]           # [B,1,S]

Distribution: data-parallel over batch. 8 cores x 4 batches each.
W/b/v replicated. Host does layout-only prep (slices + transposes +
dtype casts, no arithmetic): enc arrives per-core already k-major and
partition-blocked so every DMA descriptor is 16KB contiguous; W2^T is
ho-blocked; W1^T/hidden are bf16 (their error is seq-constant-ish and
mostly cancels in the softmax).

Per-core algorithm (big matmuls in float32r via bitcast: ~11-bit
effective mantissa at 1 cycle/row for moving dim >= 256):
    warmup: ~7 dummy MMs on zero scratch flip the PE HAM clock-gate
            to 8/8 before the real data lands.
    u      = W1^T.T @ hidden^T + b (bf16 MMs, 2MB DMA) + 8 tiny PE
             transposes -> u_all [128, 8ho, 4b]
    per chunk (sc, bi): encT [128, 8, 512] DMA'd (pre-blocked)
      T^T[ho]  = sum_kj wt[ho][kj].T @ encT[kj]     (PSUM accum, 8 mm)
      et       = tanh(T^T + u[:,ho,bi])             (ACT, bias column)
      acc     += et * v[ho]                         (DVE fused mul-add)
      pscore[bi,:] += ones-masked partition-sum of acc   (1 matmul)
      after bi=3: nmx_sc = -max(pscore); draft exp(s + nmx_sc) into
      attn_sb with running chunk sum (online softmax, ACT accum)
    tail: m = max over chunk maxes; f = exp(cmx - m); T = sum f*ssum;
          phi = f/T; attn[:, sc] *= phi[sc] (8 on DVE + 8 on ACT); DMA.
"""

import numpy as np

B, S, H = 32, 2048, 1024
NCORES = 8
BPC = B // NCORES          # batches per core
SC = 512                   # s-chunk (matmul moving size)
NSC = S // SC              # chunks per batch
KB = H // 128              # 128-blocks along one H
P = 128
NWARM = 7                  # dummy warm-up matmuls

_compiled = {}


def _build():
    import concourse.bass as bass
    import concourse.mybir as mybir
    from concourse import bacc, tile, masks

    f32 = mybir.dt.float32
    f32r = mybir.dt.float32r
    bf16 = mybir.dt.bfloat16
    Tanh = mybir.ActivationFunctionType.Tanh
    Exp = mybir.ActivationFunctionType.Exp
    Copy = mybir.ActivationFunctionType.Copy
    Mult = mybir.AluOpType.mult
    Add = mybir.AluOpType.add
    Min = mybir.AluOpType.min
    Bypass = mybir.AluOpType.bypass

    nc = bacc.Bacc("TRN2", target_bir_lowering=False, debug=False,
                   num_devices=NCORES)

    # host supplies pre-transposed/blocked layouts (layout + dtype only):
    #   enc_t: [NSC, BPC, P, KB, SC]  enc^T blocked; 16KB/partition DMAs
    #   wt2b:  [KB(ho), P, KB(kj), P] W2^T ho-col-blocked (f32r bitcast)
    #   w1cb:  [KB(ho), P, KB(kj), P] W1^T ho-col-blocked, bf16
    #   hidt:  [P, KB, BPC]           hidden^T blocked, bf16
    #   biast: [P, KB]                b blocked
    #   vt:    [P, KB]                v blocked
    enc_d = nc.declare_dram_parameter("enc_t", [NSC, BPC, P, KB, SC], f32r,
                                      isOutput=False)
    wt_d = nc.declare_dram_parameter("wt2b", [KB, P, KB, P], f32r,
                                     isOutput=False)
    w1_d = nc.declare_dram_parameter("w1cb", [KB, P, KB, P], bf16,
                                     isOutput=False)
    hidt_d = nc.declare_dram_parameter("hidt", [P, KB, BPC], bf16,
                                       isOutput=False)
    biast_d = nc.declare_dram_parameter("biast", [P, KB], f32, isOutput=False)
    vt_d = nc.declare_dram_parameter("vt", [P, KB], f32, isOutput=False)
    out_d = nc.declare_dram_parameter("attn", [BPC, S], f32, isOutput=True)

    with tile.TileContext(nc) as tc:
        import contextlib
        with contextlib.ExitStack() as ctx:
            const = ctx.enter_context(tc.tile_pool(name="const", bufs=1))
            persist = ctx.enter_context(tc.tile_pool(name="persist", bufs=1))
            wnat = ctx.enter_context(tc.tile_pool(name="wnat", bufs=2))
            encp = ctx.enter_context(tc.tile_pool(name="encp", bufs=2))
            etp = ctx.enter_context(tc.tile_pool(name="etp", bufs=3))
            accp = ctx.enter_context(tc.tile_pool(name="accp", bufs=2))
            ps_m = ctx.enter_context(
                tc.tile_pool(name="ps_m", bufs=5, space="PSUM"))
            ps_s = ctx.enter_context(
                tc.tile_pool(name="ps_s", bufs=3, space="PSUM"))

            # ---------- PE warm-up: dummy MMs on zeroed scratch ----------
            scratch = const.tile([P, SC], f32, tag="scratch")
            nc.gpsimd.memset(scratch[:], 0.0)
            pdum = ps_m.tile([P, SC], f32, tag="pm", name="pdum")
            for i in range(NWARM):
                nc.tensor.matmul(pdum[:],
                                 scratch[:, 0:P].bitcast(f32r),
                                 scratch[:].bitcast(f32r),
                                 start=(i == 0), stop=(i == NWARM - 1))

            # ---------- small constants (sync ring, first) ----------
            hidT = const.tile([P, KB, BPC], bf16, tag="hidT")
            nc.sync.dma_start(hidT[:], hidt_d[:])
            biasT = const.tile([P, KB], f32, tag="biasT")
            nc.sync.dma_start(biasT[:], biast_d[:])
            vT = const.tile([P, KB], f32, tag="vT")
            nc.sync.dma_start(vT[:], vt_d[:])

            # W2^T ho-column blocks, in ho-consumption order, split
            # across the two HWDGE rings (sync: even, scalar: odd).
            wt_ho = []
            for ho in range(KB):
                t = persist.tile([P, KB, P], f32r, tag=f"wt{ho}",
                                 name=f"wt{ho}")
                eng = nc.sync if ho % 2 == 0 else nc.scalar
                eng.dma_start(t[:], wt_d[ho])
                wt_ho.append(t)

            # ---------- enc chunk prefetch ----------
            chunks = [(sc, bi) for sc in range(NSC) for bi in range(BPC)]
            PREFETCH = 2
            pending = {}

            # chunk 0 (per-kj, gpsimd ring — interleaved with the W1
            # blocks below so the PE gets work every ~1us at startup)
            enc0 = encp.tile([P, KB, SC], f32r, tag="enc", name="enc0_0")
            # chunk 1 (per-kj, sync ring behind the even wt blocks)
            enc1 = encp.tile([P, KB, SC], f32r, tag="enc", name="enc0_1")
            for kj in range(KB):
                nc.sync.dma_start(enc1[:, kj, :], enc_d[0, 1, :, kj])

            # W1^T bf16 ho-column blocks for the u matmuls, interleaved
            # with chunk-0 pieces on the gpsimd ring.
            w1c = []
            for ho in range(KB):
                t = wnat.tile([P, KB, P], bf16, tag=f"w1c{ho}", bufs=1,
                              name=f"w1c{ho}")
                w1c.append(t)
            nc.gpsimd.dma_start(w1c[0][:], w1_d[0])
            nc.gpsimd.dma_start(w1c[1][:], w1_d[1])
            for kj in range(KB):
                nc.gpsimd.dma_start(enc0[:, kj, :], enc_d[0, 0, :, kj])
                if kj + 2 < KB:
                    nc.gpsimd.dma_start(w1c[kj + 2][:], w1_d[kj + 2])

            pending[0] = enc0
            pending[1] = enc1

            def fetch(idx):
                sc, bi = chunks[idx]
                t = encp.tile([P, KB, SC], f32r, tag="enc",
                              name=f"enc{sc}_{bi}")
                nc.gpsimd.dma_start(t[:], enc_d[sc, bi])
                return t

            # masked-ones stationaries: mask4[:, c, bi] = 1.0 iff c == bi.
            ones = const.tile([P, 1], f32, tag="ones")
            nc.gpsimd.memset(ones[:], 1.0)
            mask4 = const.tile([P, BPC, BPC], f32r, tag="mask4")
            zt = wnat.tile([P, BPC * BPC], f32, tag="zero", bufs=1)
            nc.gpsimd.memset(zt[:], 0.0)
            nc.vector.tensor_copy(
                mask4[:].rearrange("p a b -> p (a b)"), zt[:])
            for bi in range(BPC):
                nc.vector.tensor_copy(mask4[:, bi, bi:bi + 1], ones[:])

            # ---------- u^T = W1 @ hidden^T (+ bias), bf16 ----------
            # stationary = W1^T column block [128k, 128h], moving = hidT
            # [128k, 4b]; psum partition dim is h, so no transposes.
            u_all = const.tile([P, KB, BPC], f32, tag="u")
            for ho in range(KB):
                pu = ps_s.tile([P, BPC], f32, tag="ps_small", name=f"pu{ho}")
                for kj in range(KB):
                    nc.tensor.matmul(
                        pu[:], w1c[ho][:, kj, :], hidT[:, kj, :],
                        start=(kj == 0), stop=(kj == KB - 1))
                nc.vector.tensor_scalar_add(
                    u_all[:, ho, :], pu[:], biasT[:, ho:ho + 1])

            # second warm-up burst: bridges the gap between the u phase
            # and the first main chunk so the PE clock-gate stays warm
            for i in range(8):
                nc.tensor.matmul(pdum[:],
                                 scratch[:, 0:P].bitcast(f32r),
                                 scratch[:].bitcast(f32r),
                                 start=(i == 0), stop=(i == 7))

            # ---------- softmax state ----------
            attn_sb = persist.tile([BPC, S], f32, tag="attn")
            nmx = const.tile([BPC, NSC], f32, tag="nmx")     # -chunk max
            ssum = const.tile([BPC, NSC], f32, tag="ssum")   # chunk expsum
            mn = const.tile([BPC, 1], f32, tag="mn")         # -global max
            f = const.tile([BPC, NSC], f32, tag="f")         # exp(cmx - m)
            tt = const.tile([BPC, NSC], f32, tag="tt")
            t014 = const.tile([BPC, 1], f32, tag="t014")

            # ---------- main loop ----------
            for idx, (sc, bi) in enumerate(chunks):
                s0 = sc * SC
                encT = pending.pop(idx)
                if idx + PREFETCH < len(chunks):
                    pending[idx + PREFETCH] = fetch(idx + PREFETCH)

                if sc == NSC - 1 and bi == 0:
                    # precompute everything the last chunk's softmax
                    # needs, off the critical path. The last chunk's
                    # exp uses the chunks-0..14 max as stabilizer (the
                    # softmax identity is exact for any stabilizer).
                    nc.vector.tensor_reduce(
                        out=mn[:], in_=nmx[:, 0:NSC - 1], op=Min,
                        axis=mybir.AxisListType.X)
                    nc.scalar.activation(
                        f[:, 0:NSC - 1], nmx[:, 0:NSC - 1], Exp,
                        bias=mn[:], scale=-1.0)
                    nc.gpsimd.memset(f[:, NSC - 1:NSC], 1.0)
                    nc.vector.scalar_tensor_tensor(
                        tt[:, 0:NSC - 1], f[:, 0:NSC - 1], 1.0,
                        ssum[:, 0:NSC - 1],
                        op0=Bypass, op1=Mult, accum_out=t014[:])

                acc = accp.tile([P, SC], f32r, tag="acc",
                                name=f"acc{sc}_{bi}")
                for ho in range(KB):
                    pm = ps_m.tile([P, SC], f32, tag="pm",
                                   name=f"pm{sc}_{bi}_{ho}")
                    for kj in range(KB):
                        nc.tensor.matmul(
                            pm[:],
                            wt_ho[ho][:, kj, :],
                            encT[:, kj, :],
                            start=(kj == 0), stop=(kj == KB - 1))

                    et = etp.tile([P, SC], f32, tag="et",
                                  name=f"et{sc}_{bi}_{ho}")
                    nc.scalar.activation(
                        et[:], pm[:], Tanh,
                        bias=u_all[:, ho, bi:bi + 1], scale=1.0)
                    if ho == 0:
                        nc.vector.tensor_scalar_mul(
                            acc[:], et[:], vT[:, 0:1])
                    else:
                        nc.vector.scalar_tensor_tensor(
                            acc[:], et[:], vT[:, ho:ho + 1], acc[:],
                            op0=Mult, op1=Add)

                if bi == 0:
                    pscore = ps_s.tile([BPC, SC], f32, tag="ps_small",
                                       name=f"pscore{sc}")
                nc.tensor.matmul(
                    pscore[:], mask4[:, :, bi],
                    acc[:],
                    start=(bi == 0), stop=(bi == BPC - 1))
                if bi == BPC - 1:
                    # online softmax: draft exp(s - m_sc) + running sum
                    if sc < NSC - 1:
                        nc.vector.reduce_max(
                            nmx[:, sc:sc + 1], pscore[:],
                            axis=mybir.AxisListType.X, negate=True)
                        nc.scalar.activation(
                            attn_sb[:, s0:s0 + SC], pscore[:], Exp,
                            bias=nmx[:, sc:sc + 1], scale=1.0,
                            accum_out=ssum[:, sc:sc + 1])
                    else:
                        # last chunk: precomputed stabilizer, no reduce
                        nc.scalar.activation(
                            attn_sb[:, s0:s0 + SC], pscore[:], Exp,
                            bias=mn[:], scale=1.0,
                            accum_out=ssum[:, sc:sc + 1])

            # ---------- softmax tail ----------
            # T = sum f*ssum = t014 + ssum[15]; phi = f / T
            tsum = const.tile([BPC, 1], f32, tag="tsum")
            nc.vector.scalar_tensor_tensor(
                tsum[:], t014[:], 1.0, ssum[:, NSC - 1:NSC],
                op0=Bypass, op1=Add)
            rs = const.tile([BPC, 1], f32, tag="rs")
            nc.vector.reciprocal(rs[:], tsum[:])
            phi = const.tile([BPC, NSC], f32, tag="phi")
            nc.vector.tensor_scalar_mul(phi[:], f[:], rs[:])
            # rescale chunks: split across DVE and ACT queues
            HALF = NSC // 2
            for sc in range(NSC):
                s0 = sc * SC
                if sc % HALF < HALF // 2:
                    nc.vector.tensor_scalar_mul(
                        attn_sb[:, s0:s0 + SC], attn_sb[:, s0:s0 + SC],
                        phi[:, sc:sc + 1])
                else:
                    nc.scalar.activation(
                        attn_sb[:, s0:s0 + SC], attn_sb[:, s0:s0 + SC],
                        Copy, bias=0.0, scale=phi[:, sc:sc + 1])
            nc.scalar.dma_start(out_d[:], attn_sb[:])

    nc.compile()
    return nc


def _get_nc():
    if "nc" not in _compiled:
        _compiled["nc"] = _build()
    return _compiled["nc"]


def _make_in_maps(hidden, encoder_outputs, W, b, v):
    import ml_dtypes

    hidden = np.ascontiguousarray(hidden, dtype=np.float32)
    encoder_outputs = np.ascontiguousarray(encoder_outputs, dtype=np.float32)
    W = np.asarray(W, dtype=np.float32)
    b = np.asarray(b, dtype=np.float32).reshape(H)
    v = np.asarray(v, dtype=np.float32).reshape(H)

    # layout-only host prep (replicated across cores)
    WT = W.T                                                 # [2H, H]
    # W2^T blocked [ho, p(k), kj, c(h)] -> contiguous per-partition DMAs
    wt2b = np.ascontiguousarray(
        WT[H:].reshape(KB, P, KB, P).transpose(2, 1, 0, 3))  # [ho,p,kj,c]
    w1cb = np.ascontiguousarray(
        WT[:H].reshape(KB, P, KB, P).transpose(2, 1, 0, 3)
    ).astype(ml_dtypes.bfloat16)                              # [ho,p,kj,c]
    biast = np.ascontiguousarray(b.reshape(KB, P).T)          # [128, 8]
    vt = np.ascontiguousarray(v.reshape(KB, P).T)             # [128, 8]

    in_maps = []
    for c in range(NCORES):
        bs = slice(c * BPC, (c + 1) * BPC)
        hidt = np.ascontiguousarray(
            hidden[bs].T.reshape(KB, P, BPC).transpose(1, 0, 2)
        ).astype(ml_dtypes.bfloat16)                          # [128,8,4]
        # enc blocked [sc, bi, p, kj, s]; 16KB contiguous per partition
        sl = encoder_outputs[:, bs, :]                        # [S,4,H]
        enc_t = np.empty((NSC, BPC, P, KB, SC), np.float32)
        for sc in range(NSC):
            blk = sl[sc * SC:(sc + 1) * SC]                   # [512,4,1024]
            enc_t[sc] = (blk.transpose(1, 2, 0)               # [4,1024,512]
                         .reshape(BPC, KB, P, SC)
                         .transpose(0, 2, 1, 3))              # [4,128,8,512]
        in_maps.append({
            "enc_t": enc_t,
            "wt2b": wt2b,
            "w1cb": w1cb,
            "hidt": hidt,
            "biast": biast,
            "vt": vt,
        })
    return in_maps


def kernel(hidden, encoder_outputs, W, b, v):
    from concourse.bass_utils import run_bass_kernel_spmd

    nc = _get_nc()
    in_maps = _make_in_maps(hidden, encoder_outputs, W, b, v)
    res = run_bass_kernel_spmd(nc, in_maps, list(range(NCORES)))
    _compiled["last_result"] = res
    attn = np.concatenate(
        [res.results[c]["attn"] for c in range(NCORES)], axis=0)  # [B, S]
    return attn[:, None, :].astype(np.float32)


if __name__ == "__main__":
    rng = np.random.default_rng(0)
    inputs = {
        "hidden": rng.standard_normal((B, H)).astype(np.float32),
        "encoder_outputs": rng.standard_normal((S, B, H)).astype(np.float32),
        "W": (rng.standard_normal((H, 2 * H)) / np.sqrt(2 * H)).astype(np.float32),
        "b": (rng.standard_normal(H) * 0.01).astype(np.float32),
        "v": rng.standard_normal((1, H)).astype(np.float32),
    }
    out = kernel(**inputs)
    print("out", out.shape, out.dtype, out.sum())
